# revision 1
# baseline (speedup 1.0000x reference)
"""Trainium2 Bass kernel for CausalSelfAttention (q@q^T variant), 8-way
tensor-parallel over heads.

Reference semantics (B=2, T=2048, C=1024, H=16, Dh=64):
    qkv = x @ w_attn + b_attn ; q, k, v = split(qkv)      # k is UNUSED
    att = softmax(causal_mask(q @ q^T / sqrt(Dh)))         # note q@q^T (not k)
    y   = att @ v ; out = y @ w_proj + b_proj

Sharding: core c owns heads {2c, 2c+1}, both batches (tensor parallel).
After attention, an 8-way AllToAll (split in two head-halves so the first
overlaps remaining attention) redistributes y from head-sharded to
token-sharded; each core then projects (full feature dim) its own 512-row
slice of the flattened [B*T, C] output. b_proj added on host.

All matmuls run in float32r (tf32) at full PE rate.  Scores are computed
directly transposed, sT[key, query], valid because q@q^T is symmetric;
v gets a ones-column so att@v also yields the softmax denominator; causal
masking is an additive -1e30 on the PSUM scores before exp.  Emission
interleaves batch-0 attention with batch-1 qkv windows so PE/ACT work
hides the 16MB x DMA.
"""

import numpy as np

import concourse.bass as bass  # noqa: F401
import concourse.mybir as mybir
import concourse.tile as tile
from concourse import bacc
from concourse.bass_utils import run_bass_kernel_spmd
from concourse.masks import make_identity, make_upper_triangular

f32 = mybir.dt.float32
f32r = mybir.dt.float32r
bf16 = mybir.dt.bfloat16
Act = mybir.ActivationFunctionType

B, T, C, H, DH = 2, 2048, 1024, 16, 64
FT = B * T              # 4096 flat tokens
NCORES = 8
HL = 2                  # heads per core
FL = HL * DH            # 128 local features
NE = C // 128           # 8 contraction chunks
TW = 512                # qkv window
NW = FT // TW           # 8 windows
NTT = FT // 128         # 32 token tiles
TS = FT // NCORES       # 512 output rows per core
SCALE = 1.0 / 8.0       # 1/sqrt(DH)
VW = 66                 # v slot width: 64 v cols + ones col + even-pad
NEG = -1.0e30

_NC_CACHE = {}

# tuning knobs (read at build time; key the cache)
OPTS = {
    "psS_bufs": 2,
    "psO_bufs": 2,
    "eb_bufs": 4,
    "expg": 2,
    "split_a2a": True,
    "x_bf16": False,   # tf32 x-path: ~6us slower, 7x better accuracy margin
}


def build_nc(variant="spmd"):
    key = (variant, tuple(sorted(OPTS.items())))
    if key in _NC_CACHE:
        return _NC_CACHE[key]
    EXPG = OPTS["expg"]
    nc = bacc.Bacc(
        "TRN2",
        target_bir_lowering=False,
        debug=False,
        enable_asserts=True,
        num_devices=NCORES if variant == "spmd" else 1,
    )
    # f32r inputs are host-pre-rounded to tf32 bit patterns
    xdt = bf16 if OPTS.get("x_bf16", True) else f32r
    xT = nc.dram_tensor("xT", [C, FT], xdt, kind="ExternalInput")
    wq = nc.dram_tensor("wq", [C, FL], xdt, kind="ExternalInput")
    wv = nc.dram_tensor("wv", [C, FL], xdt, kind="ExternalInput")
    bq = nc.dram_tensor("bq", [FL, 1], f32, kind="ExternalInput")
    bv = nc.dram_tensor("bv", [FL, 1], f32, kind="ExternalInput")
    wp = nc.dram_tensor("wp", [C, C], f32r, kind="ExternalInput")
    out = nc.dram_tensor("out", [TS, C], f32, kind="ExternalOutput")

    with tile.TileContext(nc) as tc:
        with (
            tc.tile_pool(name="const", bufs=1) as const,
            tc.tile_pool(name="xt", bufs=2) as xpool,
            tc.tile_pool(name="vt", bufs=2) as vtpool,
            tc.tile_pool(name="eb", bufs=OPTS["eb_bufs"]) as epool,
            tc.tile_pool(name="nrm", bufs=3) as nrm,
            tc.tile_pool(name="dram", bufs=1, space="DRAM") as dpool,
        ):
            wq_sb = const.tile([128, NE, FL], xdt)
            wv_sb = const.tile([128, NE, FL], xdt)
            wp_sb = const.tile([128, NE, C], f32r)
            bq_sb = const.tile([FL, 1], f32)
            bv_sb = const.tile([FL, 1], f32)
            mask_sb = const.tile([128, 2048], f32)
            ident = const.tile([128, 128], f32)
            onz = const.tile([128, 2], f32)   # [1.0, 0.0] per partition
            qT_sb = const.tile([128, FT], f32r)           # [f_local, b*T+t]
            v_sb = const.tile([128, NTT, HL * VW], f32r)  # [t_in_tile, tile, h*VW+(d|1|pad)]
            yT_sb = const.tile([64, HL, FT], f32)         # [d, h, b*T+t]
            yTf_sb = const.tile([128, NE, TS], f32)       # post-a2a [f, chunk, t]

            # small loads first; wp spread across windows below
            nc.sync.dma_start(out=wq_sb, in_=wq.ap().rearrange("(c p) f -> p c f", p=128))
            nc.sync.dma_start(out=wv_sb, in_=wv.ap().rearrange("(c p) f -> p c f", p=128))
            nc.sync.dma_start(out=bq_sb, in_=bq.ap())
            nc.sync.dma_start(out=bv_sb, in_=bv.ap())
            make_identity(nc, ident)
            nc.vector.memset(onz[:, 0:1], 1.0)
            nc.vector.memset(onz[:, 1:2], 0.0)
            # quad causal masks, built on-chip: slot r = [0^(r*128) | triu | 1...]
            triu1 = const.tile([128, 128], f32)
            make_upper_triangular(nc, triu1, val=1.0, diag=True)
            for r in range(4):
                sl = mask_sb[:, r * 512:(r + 1) * 512]
                nc.vector.memset(sl, 1.0)
                if r:
                    nc.vector.memset(sl[:, 0:r * 128], 0.0)
                nc.vector.tensor_copy(sl[:, r * 128:(r + 1) * 128], triu1)

            _pools = {}
            xT_r = xT.ap().rearrange("(c p) t -> p c t", p=128)
            wp_r = wp.ap().rearrange("(c p) f -> p c f", p=128)

            def emit_window(w):
                """qkv for token window w: qT columns + v tiles."""
                xt = xpool.tile([128, NE, TW], xdt, tag="xt")
                nc.sync.dma_start(out=xt, in_=xT_r[:, :, w * TW:(w + 1) * TW])
                pq = _pools['psQV'].tile([128, TW], f32, tag="qv")
                for e in range(NE):
                    nc.tensor.matmul(
                        pq, lhsT=wq_sb[:, e, :], rhs=xt[:, e, :],
                        start=(e == 0), stop=(e == NE - 1),
                    )
                nc.vector.tensor_scalar_add(
                    qT_sb[:, w * TW:(w + 1) * TW], pq, bq_sb,
                )
                pv = _pools['psQV'].tile([128, TW], f32, tag="qv")
                for e in range(NE):
                    nc.tensor.matmul(
                        pv, lhsT=wv_sb[:, e, :], rhs=xt[:, e, :],
                        start=(e == 0), stop=(e == NE - 1),
                    )
                vt = vtpool.tile([128, TW], f32, tag="vt")
                nc.vector.tensor_scalar_add(vt, pv, bv_sb)
                for s in range(TW // 128):
                    tt = w * (TW // 128) + s
                    pt = _pools['psT'].tile([128, 128], f32, tag="pt")
                    nc.tensor.transpose(pt, vt[:, s * 128:(s + 1) * 128], ident)
                    dst = v_sb[:, tt, :].rearrange("p (h x) -> p h x", x=VW)
                    nc.vector.tensor_copy(
                        dst[:, :, 0:64], pt.rearrange("p (h d) -> p h d", d=DH),
                    )
                    nc.vector.tensor_copy(
                        dst[:, :, 64:66], onz.unsqueeze(1).broadcast_to((128, HL, 2)),
                    )
                # spread the 4MB wp load across windows
                nc.sync.dma_start(out=wp_sb[:, w, :], in_=wp_r[:, w, :])

            def emit_quad(h, b, qd):
                """attention for head h, batch b, query quad qd (512 queries)."""
                po = h * 64
                nj = 4 * qd + 4
                oT = _pools['psO'].tile([VW, 512], f32, tag="oT")
                rq = qT_sb[po:po + 64, b * T + qd * 512: b * T + (qd + 1) * 512]
                for g0 in range(0, nj, EXPG):
                    gs = min(EXPG, nj - g0)
                    S = _pools['psS'].tile([128, EXPG * 512], f32, tag="S")
                    for k in range(gs):
                        j = g0 + k
                        nc.tensor.matmul(
                            S[:, k * 512:(k + 1) * 512],
                            lhsT=qT_sb[po:po + 64, b * T + j * 128: b * T + (j + 1) * 128],
                            rhs=rq,
                            start=True, stop=True,
                        )
                    eb = epool.tile([128, EXPG * 512], f32r, tag="eb")
                    nc.scalar.activation(
                        eb[:, 0:gs * 512], S[:, 0:gs * 512], Act.Exp, scale=SCALE,
                    )
                    for k in range(gs):
                        j = g0 + k
                        # multiplicative 0/1 causal mask on the diagonal blocks
                        esl = eb[:, k * 512:(k + 1) * 512]
                        r = j - 4 * qd
                        if r >= 0:
                            nc.vector.tensor_mul(
                                esl, esl, mask_sb[:, r * 512:(r + 1) * 512],
                            )
                        nc.tensor.matmul(
                            oT,
                            lhsT=v_sb[:, b * (T // 128) + j, h * VW:(h + 1) * VW],
                            rhs=eb[:, k * 512:(k + 1) * 512],
                            start=(j == 0), stop=(j == nj - 1),
                        )
                rec = nrm.tile([1, 512], f32, tag="rec")
                nc.vector.reciprocal(rec, oT[64:65, :])
                recb = nrm.tile([64, 512], f32, tag="recb")
                nc.gpsimd.partition_broadcast(recb, rec)
                nc.vector.tensor_mul(
                    yT_sb[:, h, b * T + qd * 512: b * T + (qd + 1) * 512],
                    oT[0:64, :], recb,
                )

            # bounce buffers, split by head-half so a2a#1 overlaps B tail
            a2a_in = [
                dpool.tile([NCORES, 64, TS], f32, name=f"a2a_in{i}", tag=f"a2a_in{i}") for i in range(HL)
            ]
            a2a_out = [
                dpool.tile([NCORES, 64, TS], f32, name=f"a2a_out{i}", tag=f"a2a_out{i}") for i in range(HL)
            ]

            def emit_a2a(h):
                for q in range(NCORES):
                    nc.sync.dma_start(
                        out=a2a_in[h][q], in_=yT_sb[:, h, q * TS:(q + 1) * TS],
                    )
                if variant == "spmd":
                    nc.gpsimd.collective_compute(
                        "AllToAll",
                        mybir.AluOpType.bypass,
                        replica_groups=[list(range(NCORES))],
                        ins=[a2a_in[h].opt()],
                        outs=[a2a_out[h].opt()],
                    )
                else:  # timeline-estimation stand-in
                    nc.sync.dma_start(out=a2a_out[h][:], in_=a2a_in[h][:])
                # receive: head-half h -> partitions h*64..h*64+64 of each chunk
                nc.sync.dma_start(
                    out=yTf_sb[h * 64:(h + 1) * 64, :, :],
                    in_=a2a_out[h].rearrange("q d t -> d q t"),
                )
                # round to tf32 in place (DVE is a legal fp32r producer)
                nc.vector.tensor_copy(
                    yTf_sb[h * 64:(h + 1) * 64, :, :].bitcast(f32r),
                    yTf_sb[h * 64:(h + 1) * 64, :, :],
                )

            # ---------------- emission schedule ----------------
            with (
                tc.tile_pool(name="psQV", bufs=1, space="PSUM") as psQV,
                tc.tile_pool(name="psT", bufs=1, space="PSUM") as psT,
                tc.tile_pool(name="psS", bufs=OPTS["psS_bufs"], space="PSUM") as psS,
                tc.tile_pool(name="psO", bufs=OPTS["psO_bufs"], space="PSUM") as psO,
            ):
                _pools.update(psQV=psQV, psT=psT, psS=psS, psO=psO)
                NQ = OPTS.get('nq', T // 512)
                for w in range(4):                  # batch-0 windows
                    emit_window(w)
                bat0 = [(h, 0, qd) for qd in range(NQ) for h in range(HL)]
                # interleave batch-1 windows among batch-0 attention quads
                wleft = list(range(4, NW))
                for i, (h, b, qd) in enumerate(bat0):
                    if i % 2 == 0 and wleft:
                        emit_window(wleft.pop(0))
                    emit_quad(h, b, qd)
                for w in wleft:
                    emit_window(w)
                for h in range(HL):                 # batch-1 attention
                    for qd in range(NQ):
                        emit_quad(h, 1, qd)
                    if OPTS.get("do_c", True) and (OPTS["split_a2a"] or h == HL - 1):
                        emit_a2a(h)
                if OPTS.get("do_c", True) and not OPTS["split_a2a"]:
                    emit_a2a(0)

            # ---------------- output projection ----------------
            with (
                tc.tile_pool(name="psP", bufs=2, space="PSUM") as psP,
                tc.tile_pool(name="ob", bufs=2) as outpool,
            ):
                for ttile in range(TS // 128 if OPTS.get("do_d", True) else 0):
                    ob = outpool.tile([128, C], f32, tag="ob")
                    for cc in range(C // 512):
                        pp = psP.tile([128, 512], f32, tag="pp")
                        for fc in range(NE):
                            nc.tensor.matmul(
                                pp,
                                lhsT=yTf_sb.bitcast(f32r)[:, fc, ttile * 128:(ttile + 1) * 128],
                                rhs=wp_sb[:, fc, cc * 512:(cc + 1) * 512],
                                start=(fc == 0), stop=(fc == NE - 1),
                            )
                        nc.vector.tensor_copy(ob[:, cc * 512:(cc + 1) * 512], pp)
                    nc.sync.dma_start(
                        out=out.ap()[ttile * 128:(ttile + 1) * 128, :], in_=ob,
                    )

    nc.compile()
    _NC_CACHE[key] = nc
    return nc


def _round_tf32(a):
    u = np.ascontiguousarray(a, dtype=np.float32).view(np.uint32)
    r = ((u.astype(np.uint64) + 0x1000) & 0xFFFFE000).astype(np.uint32)
    return r.view(np.float32)


def make_in_maps(input_tokens, w_attn, b_attn, w_proj):
    x = np.ascontiguousarray(np.asarray(input_tokens, dtype=np.float32))
    w_attn = np.asarray(w_attn, dtype=np.float32)
    b_attn = np.asarray(b_attn, dtype=np.float32)
    w_proj = np.asarray(w_proj, dtype=np.float32)

    import ml_dtypes
    if OPTS.get("x_bf16", True):
        def xcast(a):
            return np.ascontiguousarray(a).astype(ml_dtypes.bfloat16)
    else:
        xcast = _round_tf32
    xT = xcast(np.ascontiguousarray(x.reshape(FT, C).T))  # [C, FT]
    wpr = _round_tf32(np.ascontiguousarray(w_proj))
    in_maps = []
    for c in range(NCORES):
        f0 = c * FL
        in_maps.append({
            "xT": xT,
            "wq": xcast(np.ascontiguousarray(w_attn[:, f0:f0 + FL])),
            "wv": xcast(np.ascontiguousarray(w_attn[:, 2 * C + f0:2 * C + f0 + FL])),
            "bq": np.ascontiguousarray(b_attn[f0:f0 + FL].reshape(FL, 1)),
            "bv": np.ascontiguousarray(b_attn[2 * C + f0:2 * C + f0 + FL].reshape(FL, 1)),
            "wp": wpr,
        })
    return in_maps


def assemble(results, b_proj):
    flat = np.concatenate([results[c]["out"] for c in range(NCORES)], axis=0)
    flat = flat + np.asarray(b_proj, dtype=np.float32)[None, :]
    return flat.reshape(B, T, C)


def kernel(input_tokens, w_attn, b_attn, w_proj, b_proj, _stats=None):
    nc = build_nc()
    in_maps = make_in_maps(input_tokens, w_attn, b_attn, w_proj)
    trace = _stats is not None and _stats.get("trace", False)
    try:
        res = run_bass_kernel_spmd(nc, in_maps, list(range(NCORES)), trace=trace)
    except ModuleNotFoundError:
        # NTFF profile hook unavailable in this environment
        res = run_bass_kernel_spmd(nc, in_maps, list(range(NCORES)), trace=False)
    if _stats is not None:
        _stats["exec_time_ns"] = res.exec_time_ns
        _stats["profile_json"] = res.profile_json
    return assemble(res.results, b_proj)



# revision 5
# speedup vs baseline: 1.9495x; 1.9495x over previous
"""Trainium2 Bass kernel for CausalSelfAttention (q@q^T variant), 8-way
tensor-parallel over heads.

Reference semantics (B=2, T=2048, C=1024, H=16, Dh=64):
    qkv = x @ w_attn + b_attn ; q, k, v = split(qkv)      # k is UNUSED
    att = softmax(causal_mask(q @ q^T / sqrt(Dh)))         # note q@q^T (not k)
    y   = att @ v ; out = y @ w_proj + b_proj

Sharding: core c owns heads {2c, 2c+1}, both batches (tensor parallel).
After attention, an 8-way AllToAll (split in two head-halves so the first
overlaps remaining attention) redistributes y from head-sharded to
token-sharded; each core then projects (full feature dim) its own 512-row
slice of the flattened [B*T, C] output. b_proj added on host.

All matmuls run in float32r (tf32) at full PE rate.  Scores are computed
directly transposed, sT[key, query], valid because q@q^T is symmetric;
v gets a ones-column so att@v also yields the softmax denominator; causal
masking is an additive -1e30 on the PSUM scores before exp.  Emission
interleaves batch-0 attention with batch-1 qkv windows so PE/ACT work
hides the 16MB x DMA.
"""

import numpy as np

import concourse.bass as bass  # noqa: F401
import concourse.mybir as mybir
import concourse.tile as tile
from concourse import bacc
from concourse.bass_utils import run_bass_kernel_spmd
from concourse.masks import make_identity, make_upper_triangular

f32 = mybir.dt.float32
f32r = mybir.dt.float32r
bf16 = mybir.dt.bfloat16
Act = mybir.ActivationFunctionType

B, T, C, H, DH = 2, 2048, 1024, 16, 64
FT = B * T              # 4096 flat tokens
NCORES = 8
HL = 2                  # heads per core
FL = HL * DH            # 128 local features
NE = C // 128           # 8 contraction chunks
TW = 512                # qkv window
NW = FT // TW           # 8 windows
NTT = FT // 128         # 32 token tiles
TS = FT // NCORES       # 512 output rows per core
SCALE = 1.0 / 8.0       # 1/sqrt(DH)
VW = 66                 # v slot width: 64 v cols + ones col + even-pad
NEG = -1.0e30

_NC_CACHE = {}

# tuning knobs (read at build time; key the cache)
OPTS = {
    "psS_bufs": 2,
    "psO_bufs": 2,
    "eb_bufs": 4,
    "expg": 2,
    "split_a2a": True,
    "x_bf16": False,   # tf32 x-path: ~6us slower, 7x better accuracy margin
}


def build_nc(variant="spmd"):
    key = (variant, tuple(sorted(OPTS.items())))
    if key in _NC_CACHE:
        return _NC_CACHE[key]
    EXPG = OPTS["expg"]
    nc = bacc.Bacc(
        "TRN2",
        target_bir_lowering=False,
        debug=False,
        enable_asserts=True,
        num_devices=NCORES if variant == "spmd" else 1,
    )
    # f32r inputs are host-pre-rounded to tf32 bit patterns
    xdt = bf16 if OPTS.get("x_bf16", True) else f32r
    xT = nc.dram_tensor("xT", [C, FT], xdt, kind="ExternalInput")
    wq = nc.dram_tensor("wq", [C, FL], xdt, kind="ExternalInput")
    wv = nc.dram_tensor("wv", [C, FL], xdt, kind="ExternalInput")
    bq = nc.dram_tensor("bq", [FL, 1], f32, kind="ExternalInput")
    bv = nc.dram_tensor("bv", [FL, 1], f32, kind="ExternalInput")
    wp = nc.dram_tensor("wp", [C, C], f32r, kind="ExternalInput")
    out = nc.dram_tensor("out", [TS, C], f32, kind="ExternalOutput")

    with tile.TileContext(nc) as tc:
        with (
            tc.tile_pool(name="const", bufs=1) as const,
            tc.tile_pool(name="xt", bufs=2) as xpool,
            tc.tile_pool(name="vt", bufs=2) as vtpool,
            tc.tile_pool(name="eb", bufs=OPTS["eb_bufs"]) as epool,
            tc.tile_pool(name="nrm", bufs=3) as nrm,
            tc.tile_pool(name="dram", bufs=1, space="DRAM") as dpool,
        ):
            wq_sb = const.tile([128, NE, FL], xdt)
            wv_sb = const.tile([128, NE, FL], xdt)
            wp_sb = const.tile([128, NE, C], f32r)
            bq_sb = const.tile([FL, 1], f32)
            bv_sb = const.tile([FL, 1], f32)
            mask_sb = const.tile([128, 2048], f32)
            ident = const.tile([128, 128], f32)
            onz = const.tile([128, 2], f32)   # [1.0, 0.0] per partition
            qT_sb = const.tile([128, FT], f32r)           # [f_local, b*T+t]
            v_sb = const.tile([128, NTT, HL * VW], f32r)  # [t_in_tile, tile, h*VW+(d|1|pad)]
            yT_sb = const.tile([64, HL, FT], f32)         # [d, h, b*T+t]
            yTf_sb = const.tile([128, NE, TS], f32)       # post-a2a [f, chunk, t]

            # small loads first; wp spread across windows below
            nc.sync.dma_start(out=wq_sb, in_=wq.ap().rearrange("(c p) f -> p c f", p=128))
            nc.sync.dma_start(out=wv_sb, in_=wv.ap().rearrange("(c p) f -> p c f", p=128))
            nc.sync.dma_start(out=bq_sb, in_=bq.ap())
            nc.sync.dma_start(out=bv_sb, in_=bv.ap())
            make_identity(nc, ident)
            nc.vector.memset(onz[:, 0:1], 1.0)
            nc.vector.memset(onz[:, 1:2], 0.0)
            # quad causal masks, built on-chip: slot r = [0^(r*128) | triu | 1...]
            triu1 = const.tile([128, 128], f32)
            make_upper_triangular(nc, triu1, val=1.0, diag=True)
            for r in range(4):
                sl = mask_sb[:, r * 512:(r + 1) * 512]
                nc.vector.memset(sl, 1.0)
                if r:
                    nc.vector.memset(sl[:, 0:r * 128], 0.0)
                nc.vector.tensor_copy(sl[:, r * 128:(r + 1) * 128], triu1)

            _pools = {}
            xT_r = xT.ap().rearrange("(c p) t -> p c t", p=128)
            wp_r = wp.ap().rearrange("(c p) f -> p c f", p=128)

            def emit_window(w, first=True):
                """qkv for token window w: qT columns + v tiles."""
                xt = xpool.tile([128, NE, TW], xdt, tag="xt")
                nc.sync.dma_start(out=xt, in_=xT_r[:, :, w * TW:(w + 1) * TW])
                pq = _pools['psQV'].tile([128, TW], f32, tag="qv")
                for e in range(NE):
                    nc.tensor.matmul(
                        pq, lhsT=wq_sb[:, e, :], rhs=xt[:, e, :],
                        start=(e == 0), stop=(e == NE - 1),
                    )
                nc.vector.tensor_scalar_add(
                    qT_sb[:, w * TW:(w + 1) * TW], pq, bq_sb,
                )
                pv = _pools['psQV'].tile([128, TW], f32, tag="qv")
                for e in range(NE):
                    nc.tensor.matmul(
                        pv, lhsT=wv_sb[:, e, :], rhs=xt[:, e, :],
                        start=(e == 0), stop=(e == NE - 1),
                    )
                vt = vtpool.tile([128, TW], f32, tag="vt")
                nc.vector.tensor_scalar_add(vt, pv, bv_sb)
                for s in range(TW // 128):
                    tt = w * (TW // 128) + s
                    pt = _pools['psT'].tile([128, 128], f32, tag="pt")
                    nc.tensor.transpose(pt, vt[:, s * 128:(s + 1) * 128], ident)
                    dst = v_sb[:, tt, :].rearrange("p (h x) -> p h x", x=VW)
                    nc.vector.tensor_copy(
                        dst[:, :, 0:64], pt.rearrange("p (h d) -> p h d", d=DH),
                    )
                    nc.vector.tensor_copy(
                        dst[:, :, 64:66], onz.unsqueeze(1).broadcast_to((128, HL, 2)),
                    )
                # spread the 4MB wp load across windows
                if first:
                    nc.sync.dma_start(out=wp_sb[:, w, :], in_=wp_r[:, w, :])

            def emit_quad(h, b, qd):
                """attention for head h, batch b, query quad qd (512 queries)."""
                po = h * 64
                nj = 4 * qd + 4
                oT = _pools['psO'].tile([VW, 512], f32, tag="oT")
                rq = qT_sb[po:po + 64, b * T + qd * 512: b * T + (qd + 1) * 512]
                for g0 in range(0, nj, EXPG):
                    gs = min(EXPG, nj - g0)
                    S = _pools['psS'].tile([128, EXPG * 512], f32, tag="S")
                    for k in range(gs):
                        j = g0 + k
                        nc.tensor.matmul(
                            S[:, k * 512:(k + 1) * 512],
                            lhsT=qT_sb[po:po + 64, b * T + j * 128: b * T + (j + 1) * 128],
                            rhs=rq,
                            start=True, stop=True,
                        )
                    eb = epool.tile([128, EXPG * 512], f32r, tag="eb")
                    nc.scalar.activation(
                        eb[:, 0:gs * 512], S[:, 0:gs * 512], Act.Exp, scale=SCALE,
                    )
                    for k in range(gs):
                        j = g0 + k
                        # multiplicative 0/1 causal mask on the diagonal blocks
                        esl = eb[:, k * 512:(k + 1) * 512]
                        r = j - 4 * qd
                        if r >= 0:
                            nc.vector.tensor_mul(
                                esl, esl, mask_sb[:, r * 512:(r + 1) * 512],
                            )
                        nc.tensor.matmul(
                            oT,
                            lhsT=v_sb[:, b * (T // 128) + j, h * VW:(h + 1) * VW],
                            rhs=eb[:, k * 512:(k + 1) * 512],
                            start=(j == 0), stop=(j == nj - 1),
                        )
                rec = nrm.tile([1, 512], f32, tag="rec")
                nc.vector.reciprocal(rec, oT[64:65, :])
                recb = nrm.tile([64, 512], f32, tag="recb")
                nc.gpsimd.partition_broadcast(recb, rec)
                nc.vector.tensor_mul(
                    yT_sb[:, h, b * T + qd * 512: b * T + (qd + 1) * 512],
                    oT[0:64, :], recb,
                )

            # bounce buffers, split by head-half so a2a#1 overlaps B tail
            a2a_in = [
                dpool.tile([NCORES, 64, TS], f32, name=f"a2a_in{i}", tag=f"a2a_in{i}") for i in range(HL)
            ]
            a2a_out = [
                dpool.tile([NCORES, 64, TS], f32, name=f"a2a_out{i}", tag=f"a2a_out{i}") for i in range(HL)
            ]

            def emit_a2a(h):
                for q in range(NCORES):
                    nc.sync.dma_start(
                        out=a2a_in[h][q], in_=yT_sb[:, h, q * TS:(q + 1) * TS],
                    )
                if variant == "spmd" and not OPTS.get("no_coll", False):
                    nc.gpsimd.collective_compute(
                        "AllToAll",
                        mybir.AluOpType.bypass,
                        replica_groups=[list(range(NCORES))],
                        ins=[a2a_in[h].opt()],
                        outs=[a2a_out[h].opt()],
                    )
                else:  # timeline-estimation stand-in
                    nc.sync.dma_start(out=a2a_out[h][:], in_=a2a_in[h][:])
                # receive: head-half h -> partitions h*64..h*64+64 of each chunk
                nc.sync.dma_start(
                    out=yTf_sb[h * 64:(h + 1) * 64, :, :],
                    in_=a2a_out[h].rearrange("q d t -> d q t"),
                )
                # round to tf32 in place (DVE is a legal fp32r producer)
                nc.vector.tensor_copy(
                    yTf_sb[h * 64:(h + 1) * 64, :, :].bitcast(f32r),
                    yTf_sb[h * 64:(h + 1) * 64, :, :],
                )

            for it in range(OPTS.get("iters", 1)):
                first = it == 0
                # ---------------- emission schedule ----------------
                with (
                    tc.tile_pool(name=f"psQV{it}", bufs=1, space="PSUM") as psQV,
                    tc.tile_pool(name=f"psT{it}", bufs=1, space="PSUM") as psT,
                    tc.tile_pool(name=f"psS{it}", bufs=OPTS["psS_bufs"], space="PSUM") as psS,
                    tc.tile_pool(name=f"psO{it}", bufs=OPTS["psO_bufs"], space="PSUM") as psO,
                ):
                    _pools.update(psQV=psQV, psT=psT, psS=psS, psO=psO)
                    NQ = OPTS.get('nq', T // 512)
                    for w in range(4):                  # batch-0 windows
                        emit_window(w, first)
                    bat0 = [(h, 0, qd) for qd in range(NQ) for h in range(HL)]
                    # interleave batch-1 windows among batch-0 attention quads
                    wleft = list(range(4, NW))
                    for i, (h, b, qd) in enumerate(bat0):
                        if i % 2 == 0 and wleft:
                            emit_window(wleft.pop(0), first)
                        emit_quad(h, b, qd)
                    for w in wleft:
                        emit_window(w, first)
                    for h in range(HL):                 # batch-1 attention
                        for qd in range(NQ):
                            emit_quad(h, 1, qd)
                        if OPTS.get("do_c", True) and (OPTS["split_a2a"] or h == HL - 1):
                            emit_a2a(h)
                    if OPTS.get("do_c", True) and not OPTS["split_a2a"]:
                        emit_a2a(0)

                # ---------------- output projection ----------------
                with (
                    tc.tile_pool(name=f"psP{it}", bufs=2, space="PSUM") as psP,
                    tc.tile_pool(name=f"ob{it}", bufs=2) as outpool,
                ):
                    for ttile in range(TS // 128 if OPTS.get("do_d", True) else 0):
                        ob = outpool.tile([128, C], f32, tag="ob")
                        for cc in range(C // 512):
                            pp = psP.tile([128, 512], f32, tag="pp")
                            for fc in range(NE):
                                nc.tensor.matmul(
                                    pp,
                                    lhsT=yTf_sb.bitcast(f32r)[:, fc, ttile * 128:(ttile + 1) * 128],
                                    rhs=wp_sb[:, fc, cc * 512:(cc + 1) * 512],
                                    start=(fc == 0), stop=(fc == NE - 1),
                                )
                            nc.vector.tensor_copy(ob[:, cc * 512:(cc + 1) * 512], pp)
                        nc.sync.dma_start(
                            out=out.ap()[ttile * 128:(ttile + 1) * 128, :], in_=ob,
                        )

    nc.compile()
    _NC_CACHE[key] = nc
    return nc


def _round_tf32(a):
    u = np.ascontiguousarray(a, dtype=np.float32).view(np.uint32)
    r = ((u.astype(np.uint64) + 0x1000) & 0xFFFFE000).astype(np.uint32)
    return r.view(np.float32)


def make_in_maps(input_tokens, w_attn, b_attn, w_proj):
    x = np.ascontiguousarray(np.asarray(input_tokens, dtype=np.float32))
    w_attn = np.asarray(w_attn, dtype=np.float32)
    b_attn = np.asarray(b_attn, dtype=np.float32)
    w_proj = np.asarray(w_proj, dtype=np.float32)

    import ml_dtypes
    if OPTS.get("x_bf16", True):
        def xcast(a):
            return np.ascontiguousarray(a).astype(ml_dtypes.bfloat16)
    else:
        xcast = _round_tf32
    xT = xcast(np.ascontiguousarray(x.reshape(FT, C).T))  # [C, FT]
    wpr = _round_tf32(np.ascontiguousarray(w_proj))
    in_maps = []
    for c in range(NCORES):
        f0 = c * FL
        in_maps.append({
            "xT": xT,
            "wq": xcast(np.ascontiguousarray(w_attn[:, f0:f0 + FL])),
            "wv": xcast(np.ascontiguousarray(w_attn[:, 2 * C + f0:2 * C + f0 + FL])),
            "bq": np.ascontiguousarray(b_attn[f0:f0 + FL].reshape(FL, 1)),
            "bv": np.ascontiguousarray(b_attn[2 * C + f0:2 * C + f0 + FL].reshape(FL, 1)),
            "wp": wpr,
        })
    return in_maps


def assemble(results, b_proj):
    flat = np.concatenate([results[c]["out"] for c in range(NCORES)], axis=0)
    flat = flat + np.asarray(b_proj, dtype=np.float32)[None, :]
    return flat.reshape(B, T, C)


def kernel(input_tokens, w_attn, b_attn, w_proj, b_proj, _stats=None):
    nc = build_nc()
    in_maps = make_in_maps(input_tokens, w_attn, b_attn, w_proj)
    trace = _stats is not None and _stats.get("trace", False)
    try:
        res = run_bass_kernel_spmd(nc, in_maps, list(range(NCORES)), trace=trace)
    except ModuleNotFoundError:
        # NTFF profile hook unavailable in this environment
        res = run_bass_kernel_spmd(nc, in_maps, list(range(NCORES)), trace=False)
    if _stats is not None:
        _stats["exec_time_ns"] = res.exec_time_ns
        _stats["profile_json"] = res.profile_json
    return assemble(res.results, b_proj)

